# revision 1
# baseline (speedup 1.0000x reference)
"""Trainium2 Bass kernel for nn_DistanceLoss (retrieval_knn).

Computes 5-way logits from per-tuple Euclidean distances between
frame-pair embeddings of queries and a support set.

Math restructuring vs the reference:
  - emb[n,(i,j)] = relu(A[n,i] + B[n,j] + b) with A = x@W1.T, B = x@W2.T
    (W = [W1 | W2]); frame-level matmuls are 7.5x fewer FLOPs than
    embedding each of the 120 tuples separately.
  - Tuples are enumerated gap-major ((i, i+g) for g=1..15); min and mean
    over tuples are order-invariant, and this order turns the pair
    expansion into contiguous vector adds with no broadcast APs.
  - min_u dist^2 = q^2 + min_u (s^2 - 2 q.s); support samples are
    reordered class-major on the host so each class is a contiguous
    column block.  -s^2/2 is decomposed into three e5m2 rows
    (64*hi + mid + lo, 0.02% exact) and folded into the Gram PSUM by a
    K=4 fp8-DR matmul appended to each accumulation chain; the class min
    is then a plain MAX reduction straight out of PSUM, and q^2 and the
    -2 scale fold into the final sqrt: dist = sqrt(-2*max + q^2).
  - Norms come from the DIAGONAL of tiny fp8 self-Gram matmuls
    (embedding-tile^T @ itself), extracted by a PSUM->DRAM->SBUF
    round trip that reads the dram block with a (P+1)-stride view; no
    elementwise square pass exists anywhere.

All heavy matmuls run fp8e4m3 DoubleRow (K=256 per instruction); W and
b are pre-scaled by 32 on the host so W fits fp8 dynamic range, and the
1/32 is folded into the final mean scale.  Query tuple columns are
tuple-major and packed densely into 30 stationary tiles of 128.

Sharding: queries split across 8 cores (32 each); support set, W, b
replicated.  No collectives; host concatenates logits.
"""

import sys
from contextlib import ExitStack

for _p in ("/opt/trn_rl_repo", "/root/.axon_site/_ro/trn_rl_repo"):
    if _p not in sys.path:
        sys.path.append(_p)

import ml_dtypes
import numpy as np

from concourse import bacc, mybir, tile
from concourse.bass_utils import run_bass_kernel_spmd

F32 = mybir.dt.float32
BF16 = mybir.dt.bfloat16
FP8 = mybir.dt.float8e4
FP8E5 = mybir.dt.float8e5
DR = mybir.MatmulPerfMode.DoubleRow
RELU = mybir.ActivationFunctionType.Relu
COPY = mybir.ActivationFunctionType.Copy
SQRT = mybir.ActivationFunctionType.Sqrt
MAX = mybir.AluOpType.max
ADD = mybir.AluOpType.add
SUB = mybir.AluOpType.subtract
AXX = mybir.AxisListType.X

N_CORES = 8
NQ_TOT = 256
NQC = NQ_TOT // N_CORES    # queries per core
NS = 25                    # support samples
SEQ = 16
D = 2048                   # input dim per frame
H = 1024                   # embedding dim
T = 120                    # C(16,2) frame pairs
WAY = 5
MC = H // 128              # 8 h-chunks
KC = D // 256              # 8 fp8-DR contraction chunks per W half
NT = NS * T                # 3000 support tuples
NQT = NQC * T              # 3840 query tuples
NTILE = NQT // 128         # 30 stationary query-tuple tiles
STS = 128                  # support norm tile width (se padded to 3072)
NTP = 3072                 # padded support columns for 128-wide norm tiles
NSTILE = NTP // STS
SCL = 32.0                 # host W/b scale (fp8 range); undone in final mean
BANK = 512                 # psum bank capacity in f32 columns
FQ = SEQ * NQC             # query frame columns (frame-major)
FS = NS * SEQ              # support frame columns (sample-major)

# gap-major tuple order: gap g=1..15, GOFF[g] = first tuple index of gap g
GOFF = [0, 0]
for _g in range(1, 15):
    GOFF.append(GOFF[-1] + (16 - _g))


def _bank_pieces(lo, hi):
    """Split [lo,hi) at PSUM bank boundaries (multiples of BANK)."""
    out = []
    while lo < hi:
        nxt = min(hi, (lo // BANK + 1) * BANK)
        out.append((lo, nxt))
        lo = nxt
    return out


def build_program(class_counts):
    """class_counts: support samples per class after class-major reorder."""
    bounds = [0]
    for c in class_counts:
        bounds.append(bounds[-1] + c * T)
    assert bounds[-1] == NT

    nc = bacc.Bacc("TRN2", target_bir_lowering=False, debug=False,
                   num_devices=N_CORES)

    qf_d = nc.dram_tensor("qf", [128, KC, 2, FQ], FP8,
                          kind="ExternalInput").ap()
    sf_d = nc.dram_tensor("sf", [128, KC, 2, FS], FP8,
                          kind="ExternalInput").ap()
    w_d = nc.dram_tensor("w", [128, MC, 2, KC, 2, 128], FP8,
                         kind="ExternalInput").ap()
    b_d = nc.dram_tensor("b", [128, MC], F32, kind="ExternalInput").ap()
    seg_d = nc.dram_tensor("seg", [128, NTILE, NQC], BF16,
                           kind="ExternalInput").ap()
    out_d = nc.dram_tensor("out", [NQC, WAY], F32,
                           kind="ExternalOutput").ap()
    idm_d = nc.dram_tensor("idm", [128, 128], BF16,
                           kind="ExternalInput").ap()
    fw_d = nc.dram_tensor("fw", [2, 2, 128], FP8E5,
                          kind="ExternalInput").ap()
    ide5_d = nc.dram_tensor("ide5", [128, 128], FP8E5,
                            kind="ExternalInput").ap()

    with tile.TileContext(nc) as tc, ExitStack() as top:
        cpool = top.enter_context(tc.tile_pool(name="const", bufs=1))
        epool = top.enter_context(tc.tile_pool(name="emb", bufs=1))

        segsb = cpool.tile([128, NTILE, NQC], BF16)
        nc.sync.dma_start(segsb[:, :, :], seg_d)
        bt = cpool.tile([128, MC], F32)
        nc.sync.dma_start(bt[:, :], b_d)
        foldw = cpool.tile([2, 2, 128], FP8E5)
        nc.sync.dma_start(foldw[:, :, :], fw_d)

        qe = epool.tile([128, MC, NQT], FP8)       # query tuple embeddings
        se = epool.tile([128, MC, NTP], FP8)       # support tuple embeddings
        nc.vector.memset(se[:, :, NT:NTP], 0.0)
        s2hl = epool.tile([2, 2, NT], FP8E5)       # -s^2/2 as 64*hi+mid+lo
        nc.vector.memset(s2hl[:, :, :], 0.0)
        q2sb = epool.tile([128, NTILE], F32)       # q^2 per packed tile row

        # ---- Phase F: frame matmuls (fp8 DR) + per-m tuple expansion ----
        with (
            tc.tile_pool(name="frames", bufs=1) as fpool,
            tc.tile_pool(name="wtiles", bufs=1) as wpool,
            tc.tile_pool(name="fab", bufs=1) as abpool,
            tc.tile_pool(name="pre", bufs=2) as prepool,
            tc.tile_pool(name="pf", bufs=2, space="PSUM") as pf,
        ):
            qf = fpool.tile([128, KC, 2, FQ], FP8)
            sf = fpool.tile([128, KC, 2, FS], FP8)
            wt = wpool.tile([128, MC, 2, KC, 2, 128], FP8)
            for mg in range(4):
                nc.sync.dma_start(wt[:, 2 * mg:2 * mg + 2],
                                  w_d[:, 2 * mg:2 * mg + 2])
            nc.sync.dma_start(qf[:, :, :, :], qf_d)
            nc.sync.dma_start(sf[:, :, :, :], sf_d)

            # query frames frame-major, flat for 1-D expansion slices
            qAB = abpool.tile([128, MC * 2 * FQ], BF16)
            # support frames sample-major: [128, MC, half, NS, SEQ]
            sAB = abpool.tile([128, MC, 2, NS, SEQ], BF16)

            def emit_relu(m, qp, sp_):
                nc.scalar.activation(qe[:, m], qp[:, :], RELU,
                                     bias=bt[:, m:m + 1], scale=1.0)
                nc.scalar.activation(se[:, m, 0:NT], sp_[:, :, :], RELU,
                                     bias=bt[:, m:m + 1], scale=1.0)

            relu_pend = None
            for m in range(MC):
                for half in range(2):
                    pq = pf.tile([128, FQ], F32, tag="pq")
                    ps = pf.tile([128, FS], F32, tag="ps")
                    for k in range(KC):
                        st, sp = k == 0, k == KC - 1
                        nc.tensor.matmul(pq[:, :], wt[:, m, half, k],
                                         qf[:, k], start=st, stop=sp,
                                         perf_mode=DR)
                        nc.tensor.matmul(ps[:, :], wt[:, m, half, k],
                                         sf[:, k], start=st, stop=sp,
                                         perf_mode=DR)
                    nc.scalar.copy(
                        qAB[:, (2 * m + half) * FQ:(2 * m + half + 1) * FQ],
                        pq[:, :])
                    nc.scalar.copy(sAB[:, m, half], ps[:, :])
                # previous m's relu goes behind this m's copies on the
                # scalar queue so copies never wait on expansion
                if relu_pend is not None:
                    emit_relu(*relu_pend)
                # gap-major expansion: tuples (i, i+g)
                qpre = prepool.tile([128, NQT], BF16, tag="qpre")
                spre = prepool.tile([128, NS, T], BF16, tag="spre")
                a0 = 2 * m * FQ
                b0 = (2 * m + 1) * FQ
                for g in range(1, SEQ):
                    n = SEQ - g
                    # query side: purely 1-D contiguous slices
                    nc.vector.tensor_tensor(
                        out=qpre[:, GOFF[g] * NQC:(GOFF[g] + n) * NQC],
                        in0=qAB[:, a0:a0 + n * NQC],
                        in1=qAB[:, b0 + g * NQC:b0 + SEQ * NQC], op=ADD)
                    eng = nc.gpsimd if g <= 9 else nc.vector
                    eng.tensor_tensor(
                        out=spre[:, :, GOFF[g]:GOFF[g] + n],
                        in0=sAB[:, m, 0, :, 0:n],
                        in1=sAB[:, m, 1, :, g:SEQ], op=ADD)
                relu_pend = (m, qpre, spre)
            emit_relu(*relu_pend)

        # ---- Phase N: norms from self-Gram diagonals ----
        with (
            tc.tile_pool(name="nsb", bufs=1) as nsb,
            tc.tile_pool(name="dg", bufs=2) as dg,
            tc.tile_pool(name="pn", bufs=2, space="PSUM") as pn,
            tc.tile_pool(name="pt", bufs=1, space="PSUM") as pt,
        ):
            idm = nsb.tile([128, 128], BF16)
            nc.sync.dma_start(idm[:, :], idm_d)
            ide5 = nsb.tile([128, 128], FP8E5)
            nc.sync.dma_start(ide5[:, :], ide5_d)
            s2c = nsb.tile([STS, NSTILE], F32)

            def norm_tiles(src, ntl, w, dst):
                # diag(tile^T @ tile) = column norms; extract by
                # identity-mask multiply + row-sum
                for t in range(ntl):
                    pq2 = pn.tile([128, 128], F32, tag="pq2")
                    for kc in range(MC // 2):
                        lhs = src[:, 2 * kc:2 * kc + 2, w * t:w * (t + 1)]
                        nc.tensor.matmul(pq2[0:w, 0:w], lhs, lhs,
                                         start=(kc == 0),
                                         stop=(kc == MC // 2 - 1),
                                         perf_mode=DR)
                    dsc = dg.tile([128, 128], F32, tag="dsc")
                    nc.vector.tensor_tensor(out=dsc[0:w, 0:w],
                                            in0=pq2[0:w, 0:w],
                                            in1=idm[0:w, 0:w],
                                            op=mybir.AluOpType.mult)
                    nc.vector.tensor_reduce(dst[0:w, t:t + 1],
                                            dsc[0:w, 0:w],
                                            axis=AXX, op=ADD)

            norm_tiles(se, NSTILE, STS, s2c)
            # -s^2/2 = 64*hi + mid + lo in e5m2, computed columnwise
            hml = nsb.tile([STS, 3, NSTILE], FP8E5)
            r1 = nsb.tile([STS, NSTILE], F32)
            tmp = nsb.tile([STS, NSTILE], F32)
            nc.scalar.activation(hml[:, 0, :], s2c[:, :], COPY,
                                 scale=-1.0 / 128.0)
            nc.scalar.activation(tmp[:, :], hml[:, 0, :], COPY, scale=64.0)
            nc.scalar.activation(r1[:, :], s2c[:, :], COPY, scale=-0.5)
            nc.vector.tensor_tensor(out=r1[:, :], in0=r1[:, :],
                                    in1=tmp[:, :], op=SUB)
            nc.scalar.copy(hml[:, 1, :], r1[:, :])
            nc.scalar.copy(tmp[:, :], hml[:, 1, :])
            nc.vector.tensor_tensor(out=r1[:, :], in0=r1[:, :],
                                    in1=tmp[:, :], op=SUB)
            nc.scalar.copy(hml[:, 2, :], r1[:, :])
            # e5m2 values are exactly representable in bf16, so convert,
            # transpose in bf16, and round back (lossless round trip)
            hmlb = nsb.tile([STS, 3, NSTILE], BF16)
            nc.scalar.copy(hmlb[:, :, :], hml[:, :, :])
            norm_tiles(qe, NTILE, 128, q2sb)
            pt3 = pt.tile([3, NSTILE * STS], BF16)
            for t in range(NSTILE):
                nc.tensor.matmul(pt3[:, STS * t:STS * (t + 1)],
                                 hmlb[:, :, t], idm[:, :],
                                 is_transpose=True)
            stg = nsb.tile([3, NSTILE * STS], FP8E5)
            nc.scalar.copy(stg[:, :], pt3[:, :])
            nc.sync.dma_start(s2hl[0:1, 0, :], stg[0:1, 0:NT])
            nc.scalar.dma_start(s2hl[0:1, 1, :], stg[1:2, 0:NT])
            nc.gpsimd.dma_start(s2hl[1:2, 0, :], stg[2:3, 0:NT])

        # ---- Phase G: Gram (+e5m2 fold) + class max + sqrt + mean ----
        chunks = _bank_pieces(0, NT)
        cls_pieces = [_bank_pieces(bounds[c], bounds[c + 1])
                      for c in range(WAY)]
        uniform2 = all(len(p) == 2 for p in cls_pieces)
        with (
            tc.tile_pool(name="gps", bufs=1, space="PSUM") as gp,
            tc.tile_pool(name="mps", bufs=1, space="PSUM") as mp,
            tc.tile_pool(name="dts", bufs=1) as dpool,
            tc.tile_pool(name="acc", bufs=2) as apool,
        ):
            gb = [gp.tile([128, c1 - c0], F32, name=f"gb{ci}")
                  for ci, (c0, c1) in enumerate(chunks)]
            mpsum = mp.tile([NQC, NTILE, WAY], F32)
            dtsb = dpool.tile([128, NTILE, WAY], BF16)

            for t in range(NTILE):
                bks = list(range(len(chunks)))
                for ci, (c0, c1) in enumerate(chunks):
                    for kc in range(MC // 2):
                        nc.tensor.matmul(
                            gb[ci][:, :],
                            qe[:, 2 * kc:2 * kc + 2, 128 * t:128 * (t + 1)],
                            se[:, 2 * kc:2 * kc + 2, c0:c1],
                            start=(kc == 0), stop=False,
                            perf_mode=DR)
                    # fold -s^2/2 (64*hi + mid + lo), staying in fp8 DR
                    nc.tensor.matmul(gb[ci][:, :], foldw[:, :, :],
                                     s2hl[:, :, c0:c1],
                                     start=False, stop=True, perf_mode=DR)
                # class max of (g - s^2/2) straight from PSUM bank pieces
                mp2 = apool.tile([128, WAY, 2], F32, tag="mp2")
                maxacc = apool.tile([128, WAY], F32, tag="acc")
                for cls in range(WAY):
                    pieces = cls_pieces[cls]
                    if uniform2:
                        for pi, (p0, p1) in enumerate(pieces):
                            ci = p0 // BANK
                            b0 = p0 - chunks[ci][0]
                            nc.vector.tensor_reduce(
                                mp2[:, cls, pi:pi + 1],
                                gb[bks[ci]][:, b0:b0 + p1 - p0],
                                axis=AXX, op=MAX)
                    else:
                        if not pieces:
                            nc.vector.memset(maxacc[:, cls:cls + 1], -3.0e38)
                            continue
                        for pi, (p0, p1) in enumerate(pieces):
                            ci = p0 // BANK
                            b0 = p0 - chunks[ci][0]
                            dst = (maxacc[:, cls:cls + 1] if pi == 0
                                   else mp2[:, 0, 0:1])
                            nc.vector.tensor_reduce(
                                dst, gb[bks[ci]][:, b0:b0 + p1 - p0],
                                axis=AXX, op=MAX)
                            if pi > 0:
                                nc.vector.tensor_tensor(
                                    out=maxacc[:, cls:cls + 1],
                                    in0=maxacc[:, cls:cls + 1],
                                    in1=mp2[:, 0, 0:1], op=MAX)
                if uniform2:
                    nc.vector.tensor_reduce(maxacc[:, :], mp2[:, :, :],
                                            axis=AXX, op=MAX)
                # dist = sqrt(-2*max + q^2)
                nc.scalar.activation(dtsb[:, t, :], maxacc[:, :], SQRT,
                                     bias=q2sb[:, t:t + 1], scale=-2.0)

            for t in range(NTILE):
                nc.tensor.matmul(mpsum[:, t, :], segsb[:, t, :],
                                 dtsb[:, t, :], start=True, stop=True)
            plog = apool.tile([NQC, WAY], F32, tag="plog")
            for cls in range(WAY):
                nc.vector.tensor_reduce(plog[:, cls:cls + 1],
                                        mpsum[:, :, cls], axis=AXX, op=ADD)
            louts = apool.tile([NQC, WAY], F32, tag="louts")
            nc.scalar.activation(louts[:, :], plog[:, :], COPY,
                                 scale=-1.0 / (T * SCL))
            nc.sync.dma_start(out_d, louts[:, :])
    nc.compile()
    return nc


_NC_CACHE = {}
LAST = None


def _frames_fp8(x, n, frame_major):
    """[n, SEQ, D] f32 -> [128, KC, 2, cols] fp8 (DR moving layout)."""
    f8 = ml_dtypes.float8_e4m3
    if frame_major:
        fr = x.transpose(1, 0, 2).reshape(SEQ * n, KC, 2, 128)
    else:
        fr = x.reshape(n * SEQ, KC, 2, 128)
    return np.ascontiguousarray(fr.transpose(3, 1, 2, 0).astype(f8))


def kernel(support_set, queries, support_labels, W, b):
    global LAST
    support_set = np.asarray(support_set, dtype=np.float32)
    queries = np.asarray(queries, dtype=np.float32)
    support_labels = np.asarray(support_labels)
    W = np.asarray(W, dtype=np.float32)
    b = np.asarray(b, dtype=np.float32)
    f8 = ml_dtypes.float8_e4m3

    # class-major support reorder (class blocks contiguous)
    perm = np.argsort(support_labels, kind="stable")
    counts = tuple(int((support_labels == c).sum()) for c in range(WAY))
    sf = _frames_fp8(support_set[perm], NS, frame_major=False)

    # W: [p, m, half, kc, pair, hcol], scaled into fp8 range
    wt = np.ascontiguousarray(
        (W * SCL).reshape(MC, 128, 2, KC, 2, 128)
        .transpose(5, 0, 2, 3, 4, 1).astype(f8))
    bt = np.ascontiguousarray((b * SCL).reshape(MC, 128).T)

    # segment matrix: query-tuple cols are tuple-major -> query = col % NQC
    seg = np.zeros((128, NTILE, NQC), dtype=np.float32)
    for t in range(NTILE):
        for r in range(128):
            seg[r, t, (128 * t + r) % NQC] = 1.0
    seg = seg.astype(ml_dtypes.bfloat16)
    idm = np.eye(128, dtype=np.float32).astype(ml_dtypes.bfloat16)
    fw = np.zeros((2, 2, 128), dtype=np.float32)
    fw[0, 0] = 64.0
    fw[0, 1] = 1.0
    fw[1, 0] = 1.0
    fw = fw.astype(ml_dtypes.float8_e5m2)
    ide5 = np.eye(128, dtype=np.float32).astype(ml_dtypes.float8_e5m2)

    in_maps = []
    for c in range(N_CORES):
        qfc = _frames_fp8(queries[c * NQC:(c + 1) * NQC], NQC,
                          frame_major=True)
        in_maps.append({"qf": qfc, "sf": sf, "w": wt, "b": bt, "seg": seg,
                        "idm": idm, "fw": fw, "ide5": ide5})

    if counts not in _NC_CACHE:
        _NC_CACHE[counts] = build_program(counts)
    res = run_bass_kernel_spmd(_NC_CACHE[counts], in_maps,
                               list(range(N_CORES)))
    LAST = res
    outs = [res.results[c]["out"] for c in range(N_CORES)]
    return np.concatenate(outs, axis=0)


if __name__ == "__main__":
    rng = np.random.default_rng(0)
    out = kernel(
        rng.standard_normal((NS, SEQ, D)).astype(np.float32),
        rng.standard_normal((NQ_TOT, SEQ, D)).astype(np.float32),
        (np.arange(NS) % WAY).astype(np.int32),
        (rng.standard_normal((H, 2 * D)) / np.sqrt(2 * D)).astype(np.float32),
        (rng.standard_normal(H) * 0.01).astype(np.float32),
    )
    print(out.shape, out[:2])



# revision 15
# speedup vs baseline: 1.3248x; 1.3248x over previous
"""Trainium2 Bass kernel for nn_DistanceLoss (retrieval_knn).

Computes 5-way logits from per-tuple Euclidean distances between
frame-pair embeddings of queries and a support set.

Math restructuring vs the reference:
  - emb[n,(i,j)] = relu(A[n,i] + B[n,j] + b) with A = x@W1.T, B = x@W2.T
    (W = [W1 | W2]); frame-level matmuls are 7.5x fewer FLOPs than
    embedding each of the 120 tuples separately.
  - Tuples are enumerated gap-major ((i, i+g) for g=1..15); min and mean
    over tuples are order-invariant, and this order turns the pair
    expansion into contiguous vector adds with no broadcast APs.
  - min_u dist^2 = q^2 + min_u (s^2 - 2 q.s); support samples are
    reordered class-major on the host so each class is a contiguous
    column block.  -s^2/2 is materialized once as a [128, NT] f32 SBUF
    tensor (broadcast across partitions by a tiny ones-stationary
    matmul, hi+lo bf16 split for accuracy); the scalar engine, idle
    during the Gram phase, seeds each PSUM bank with it before the
    fp8-DR accumulation chain (start=False), so the PE streams no fold
    columns at all and never switches dtypes.  The class min is then a
    plain MAX reduction straight out of PSUM, and q^2 and the -2 scale
    fold into the final sqrt: dist = sqrt(-2*max + q^2).
  - Norms come from the DIAGONAL of tiny fp8 self-Gram matmuls
    (embedding-tile^T @ itself), extracted by a PSUM->DRAM->SBUF
    round trip that reads the dram block with a (P+1)-stride view; no
    elementwise square pass exists anywhere.

All heavy matmuls run fp8e4m3 DoubleRow (K=256 per instruction); W and
b are pre-scaled by 32 on the host so W fits fp8 dynamic range, and the
1/32 is folded into the final mean scale.  Query tuple columns are
tuple-major and packed densely into 30 stationary tiles of 128.

Sharding: queries split across 8 cores (32 each); support set, W, b
replicated.  No collectives; host concatenates logits.
"""

import sys
from contextlib import ExitStack

for _p in ("/opt/trn_rl_repo", "/root/.axon_site/_ro/trn_rl_repo"):
    if _p not in sys.path:
        sys.path.append(_p)

import ml_dtypes
import numpy as np

from concourse import bacc, mybir, tile
from concourse.bass_utils import run_bass_kernel_spmd

F32 = mybir.dt.float32
BF16 = mybir.dt.bfloat16
FP8 = mybir.dt.float8e4
FP8E5 = mybir.dt.float8e5
DR = mybir.MatmulPerfMode.DoubleRow
RELU = mybir.ActivationFunctionType.Relu
COPY = mybir.ActivationFunctionType.Copy
SQRT = mybir.ActivationFunctionType.Sqrt
MAX = mybir.AluOpType.max
ADD = mybir.AluOpType.add
SUB = mybir.AluOpType.subtract
AXX = mybir.AxisListType.X

N_CORES = 8
NQ_TOT = 256
NQC = NQ_TOT // N_CORES    # queries per core
NS = 25                    # support samples
SEQ = 16
D = 2048                   # input dim per frame
H = 1024                   # embedding dim
T = 120                    # C(16,2) frame pairs
WAY = 5
MC = H // 128              # 8 h-chunks
KC = D // 256              # 8 fp8-DR contraction chunks per W half
NT = NS * T                # 3000 support tuples
NQT = NQC * T              # 3840 query tuples
NTILE = NQT // 128         # 30 stationary query-tuple tiles
STS = 128                  # support norm tile width (se padded to 3072)
NTP = 3072                 # padded support columns for 128-wide norm tiles
NSTILE = NTP // STS
SCL = 32.0                 # host W/b scale (fp8 range); undone in final mean
BANK = 512                 # psum bank capacity in f32 columns
FQ = SEQ * NQC             # query frame columns (frame-major)
FS = NS * SEQ              # support frame columns (sample-major)

# gap-major tuple order: gap g=1..15, GOFF[g] = first tuple index of gap g
GOFF = [0, 0]
for _g in range(1, 15):
    GOFF.append(GOFF[-1] + (16 - _g))


def _bank_pieces(lo, hi):
    """Split [lo,hi) at PSUM bank boundaries (multiples of BANK)."""
    out = []
    while lo < hi:
        nxt = min(hi, (lo // BANK + 1) * BANK)
        out.append((lo, nxt))
        lo = nxt
    return out


DEBUG = False


def build_program(class_counts):
    """class_counts: support samples per class after class-major reorder."""
    bounds = [0]
    for c in class_counts:
        bounds.append(bounds[-1] + c * T)
    assert bounds[-1] == NT

    nc = bacc.Bacc("TRN2", target_bir_lowering=False, debug=False,
                   num_devices=N_CORES)

    qf_d = nc.dram_tensor("qf", [128, KC, 2, FQ], FP8,
                          kind="ExternalInput").ap()
    sf_d = nc.dram_tensor("sf", [128, KC, 2, FS], FP8,
                          kind="ExternalInput").ap()
    w_d = nc.dram_tensor("w", [128, MC, 2, KC, 2, 128], FP8,
                         kind="ExternalInput").ap()
    b_d = nc.dram_tensor("b", [128, MC], F32, kind="ExternalInput").ap()
    seg_d = nc.dram_tensor("seg", [128, NTILE, NQC], BF16,
                           kind="ExternalInput").ap()
    out_d = nc.dram_tensor("out", [NQC, WAY], F32,
                           kind="ExternalOutput").ap()
    idm_d = nc.dram_tensor("idm", [128, 128], BF16,
                           kind="ExternalInput").ap()
    if DEBUG:
        cdb_d = nc.dram_tensor("cdb", [128, NT], F32,
                               kind="ExternalOutput").ap()
        sdb_d = nc.dram_tensor("sdb", [128, NSTILE], F32,
                               kind="ExternalOutput").ap()

    with tile.TileContext(nc) as tc, ExitStack() as top:
        cpool = top.enter_context(tc.tile_pool(name="const", bufs=1))
        epool = top.enter_context(tc.tile_pool(name="emb", bufs=1))

        segsb = cpool.tile([128, NTILE, NQC], BF16)
        nc.sync.dma_start(segsb[:, :, :], seg_d)
        bt = cpool.tile([128, MC], F32)
        nc.sync.dma_start(bt[:, :], b_d)

        qe = epool.tile([128, MC, NQT], FP8)       # query tuple embeddings
        se = epool.tile([128, MC, NTP], FP8)       # support tuple embeddings
        nc.vector.memset(se[:, :, NT:NTP], 0.0)
        cbig = epool.tile([128, NT], F32)          # -s^2/2 bcast to 128 rows
        crow = epool.tile([2, NSTILE * STS], BF16)  # -s^2/2 as hi/lo rows
        ones2 = epool.tile([2, 128], BF16)
        nc.vector.memset(ones2[:, :], 1.0)
        q2sb = epool.tile([128, NTILE], F32)       # q^2 per packed tile row

        # ---- Phase F: frame matmuls (fp8 DR) + per-m tuple expansion ----
        with (
            tc.tile_pool(name="frames", bufs=1) as fpool,
            tc.tile_pool(name="wtiles", bufs=1) as wpool,
            tc.tile_pool(name="fab", bufs=1) as abpool,
            tc.tile_pool(name="pre", bufs=2) as prepool,
            tc.tile_pool(name="pf", bufs=2, space="PSUM") as pf,
        ):
            qf = fpool.tile([128, KC, 2, FQ], FP8)
            sf = fpool.tile([128, KC, 2, FS], FP8)
            wt = wpool.tile([128, MC, 2, KC, 2, 128], FP8)
            for mg in range(4):
                nc.sync.dma_start(wt[:, 2 * mg:2 * mg + 2],
                                  w_d[:, 2 * mg:2 * mg + 2])
            nc.sync.dma_start(qf[:, :, :, :], qf_d)
            nc.sync.dma_start(sf[:, :, :, :], sf_d)

            # query frames frame-major, flat for 1-D expansion slices
            qAB = abpool.tile([128, MC * 2 * FQ], BF16)
            # support frames sample-major: [128, MC, half, NS, SEQ]
            sAB = abpool.tile([128, MC, 2, NS, SEQ], BF16)

            def emit_relu(m, qp, sp_):
                # support first: the norm/correction chain hangs off se
                nc.scalar.activation(se[:, m, 0:NT], sp_[:, :, :], RELU,
                                     bias=bt[:, m:m + 1], scale=1.0)
                nc.scalar.activation(qe[:, m], qp[:, :], RELU,
                                     bias=bt[:, m:m + 1], scale=1.0)

            relu_pend = None
            for m in range(MC):
                for half in range(2):
                    pq = pf.tile([128, FQ], F32, tag="pq")
                    ps = pf.tile([128, FS], F32, tag="ps")
                    for k in range(KC):
                        st, sp = k == 0, k == KC - 1
                        nc.tensor.matmul(pq[:, :], wt[:, m, half, k],
                                         qf[:, k], start=st, stop=sp,
                                         perf_mode=DR)
                        nc.tensor.matmul(ps[:, :], wt[:, m, half, k],
                                         sf[:, k], start=st, stop=sp,
                                         perf_mode=DR)
                    nc.scalar.copy(
                        qAB[:, (2 * m + half) * FQ:(2 * m + half + 1) * FQ],
                        pq[:, :])
                    nc.scalar.copy(sAB[:, m, half], ps[:, :])
                # previous m's relu goes behind this m's copies on the
                # scalar queue so copies never wait on expansion
                if relu_pend is not None:
                    emit_relu(*relu_pend)
                # gap-major expansion: tuples (i, i+g)
                qpre = prepool.tile([128, NQT], BF16, tag="qpre")
                spre = prepool.tile([128, NS, T], BF16, tag="spre")
                a0 = 2 * m * FQ
                b0 = (2 * m + 1) * FQ
                for g in range(1, SEQ):
                    n = SEQ - g
                    # query side: purely 1-D contiguous slices
                    nc.vector.tensor_tensor(
                        out=qpre[:, GOFF[g] * NQC:(GOFF[g] + n) * NQC],
                        in0=qAB[:, a0:a0 + n * NQC],
                        in1=qAB[:, b0 + g * NQC:b0 + SEQ * NQC], op=ADD)
                    # gpsimd is ~5x slower per element than DVE's 2x bf16
                    # path; keep all expansion adds on the vector engine
                    nc.vector.tensor_tensor(
                        out=spre[:, :, GOFF[g]:GOFF[g] + n],
                        in0=sAB[:, m, 0, :, 0:n],
                        in1=sAB[:, m, 1, :, g:SEQ], op=ADD)
                relu_pend = (m, qpre, spre)
            emit_relu(*relu_pend)

        # ---- Phase N: norms from self-Gram diagonals ----
        with (
            tc.tile_pool(name="nsb", bufs=1) as nsb,
            tc.tile_pool(name="dg", bufs=2) as dg,
            tc.tile_pool(name="pn", bufs=2, space="PSUM") as pn,
            tc.tile_pool(name="pt", bufs=1, space="PSUM") as pt,
        ):
            idm = nsb.tile([128, 128], BF16)
            nc.sync.dma_start(idm[:, :], idm_d)
            s2c = nsb.tile([STS, NSTILE], F32)

            def norm_tiles(src, ntl, w, dst):
                # diag(tile^T @ tile) = column norms; extract by
                # identity-mask multiply + row-sum
                for t in range(ntl):
                    pq2 = pn.tile([128, 128], F32, tag="pq2")
                    for kc in range(MC // 2):
                        lhs = src[:, 2 * kc:2 * kc + 2, w * t:w * (t + 1)]
                        nc.tensor.matmul(pq2[0:w, 0:w], lhs, lhs,
                                         start=(kc == 0),
                                         stop=(kc == MC // 2 - 1),
                                         perf_mode=DR)
                    dsc = dg.tile([128, 128], F32, tag="dsc")
                    nc.vector.tensor_tensor(out=dsc[0:w, 0:w],
                                            in0=pq2[0:w, 0:w],
                                            in1=idm[0:w, 0:w],
                                            op=mybir.AluOpType.mult)
                    nc.vector.tensor_reduce(dst[0:w, t:t + 1],
                                            dsc[0:w, 0:w],
                                            axis=AXX, op=ADD)

            norm_tiles(se, NSTILE, STS, s2c)
            # -s^2/2 as bf16 hi + lo (Dekker split: exact to ~16 bits)
            h2 = nsb.tile([STS, 2, NSTILE], BF16)
            r1 = nsb.tile([STS, NSTILE], F32)
            tmp = nsb.tile([STS, NSTILE], F32)
            nc.scalar.activation(h2[:, 0, :], s2c[:, :], COPY, scale=-0.5)
            nc.scalar.activation(r1[:, :], s2c[:, :], COPY, scale=-0.5)
            nc.scalar.copy(tmp[:, :], h2[:, 0, :])
            nc.vector.tensor_tensor(out=r1[:, :], in0=r1[:, :],
                                    in1=tmp[:, :], op=SUB)
            nc.scalar.copy(h2[:, 1, :], r1[:, :])
            norm_tiles(qe, NTILE, 128, q2sb)
            # transpose hi/lo to rows; the broadcast to 128 partitions
            # happens in phase G (priming the gram banks)
            pt2 = pt.tile([2, NSTILE * STS], BF16)
            for t in range(NSTILE):
                nc.tensor.matmul(pt2[:, STS * t:STS * (t + 1)],
                                 h2[:, :, t], idm[:, :],
                                 is_transpose=True)
            nc.scalar.copy(crow[:, :], pt2[:, :])
            if DEBUG:
                nc.sync.dma_start(sdb_d, s2c[:, :])

        # ---- Phase G: Gram (+e5m2 fold) + class max + sqrt + mean ----
        chunks = _bank_pieces(0, NT)
        cls_pieces = [_bank_pieces(bounds[c], bounds[c + 1])
                      for c in range(WAY)]
        uniform2 = all(len(p) == 2 for p in cls_pieces)
        with (
            tc.tile_pool(name="gps", bufs=1, space="PSUM") as gp,
            tc.tile_pool(name="mps", bufs=1, space="PSUM") as mp,
            tc.tile_pool(name="dts", bufs=1) as dpool,
            tc.tile_pool(name="acc", bufs=2) as apool,
        ):
            gb = [gp.tile([128, c1 - c0], F32, name=f"gb{ci}")
                  for ci, (c0, c1) in enumerate(chunks)]
            mpsum = mp.tile([NQC, NTILE, WAY], F32)
            dtsb = dpool.tile([128, NTILE, WAY], BF16)

            # Prime each bank with a closed bcast chain writing -s^2/2
            # (a start=False chain only accumulates onto PSUM memory
            # after the bank has seen a stopped chain), and stash the
            # broadcast rows to SBUF for the later per-tile reseeds.
            for ci, (c0, c1) in enumerate(chunks):
                nc.tensor.matmul(gb[ci][:, :], ones2[:, :],
                                 crow[:, c0:c1], start=True, stop=True)
                nc.scalar.copy(cbig[:, c0:c1], gb[ci][:, :])
            if DEBUG:
                nc.sync.dma_start(cdb_d, cbig[:, :])

            for t in range(NTILE):
                bks = list(range(len(chunks)))
                for ci, (c0, c1) in enumerate(chunks):
                    # scalar engine (idle here) reseeds the bank with
                    # -s^2/2; the whole fp8 chain then accumulates on
                    # top (start=False) and the PE streams no fold cols
                    if t > 0:
                        nc.scalar.copy(gb[ci][:, :], cbig[:, c0:c1])
                    for kc in range(MC // 2):
                        nc.tensor.matmul(
                            gb[ci][:, :],
                            qe[:, 2 * kc:2 * kc + 2, 128 * t:128 * (t + 1)],
                            se[:, 2 * kc:2 * kc + 2, c0:c1],
                            start=False, stop=(kc == MC // 2 - 1),
                            perf_mode=DR)
                # class max of (g - s^2/2) straight from PSUM bank pieces
                mp2 = apool.tile([128, WAY, 2], F32, tag="mp2")
                maxacc = apool.tile([128, WAY], F32, tag="acc")
                for cls in range(WAY):
                    pieces = cls_pieces[cls]
                    if uniform2:
                        for pi, (p0, p1) in enumerate(pieces):
                            ci = p0 // BANK
                            b0 = p0 - chunks[ci][0]
                            nc.vector.tensor_reduce(
                                mp2[:, cls, pi:pi + 1],
                                gb[bks[ci]][:, b0:b0 + p1 - p0],
                                axis=AXX, op=MAX)
                    else:
                        if not pieces:
                            nc.vector.memset(maxacc[:, cls:cls + 1], -3.0e38)
                            continue
                        for pi, (p0, p1) in enumerate(pieces):
                            ci = p0 // BANK
                            b0 = p0 - chunks[ci][0]
                            dst = (maxacc[:, cls:cls + 1] if pi == 0
                                   else mp2[:, 0, 0:1])
                            nc.vector.tensor_reduce(
                                dst, gb[bks[ci]][:, b0:b0 + p1 - p0],
                                axis=AXX, op=MAX)
                            if pi > 0:
                                nc.vector.tensor_tensor(
                                    out=maxacc[:, cls:cls + 1],
                                    in0=maxacc[:, cls:cls + 1],
                                    in1=mp2[:, 0, 0:1], op=MAX)
                if uniform2:
                    nc.vector.tensor_reduce(maxacc[:, :], mp2[:, :, :],
                                            axis=AXX, op=MAX)
                # dist = sqrt(-2*max + q^2)
                nc.scalar.activation(dtsb[:, t, :], maxacc[:, :], SQRT,
                                     bias=q2sb[:, t:t + 1], scale=-2.0)

            for t in range(NTILE):
                nc.tensor.matmul(mpsum[:, t, :], segsb[:, t, :],
                                 dtsb[:, t, :], start=True, stop=True)
            plog = apool.tile([NQC, WAY], F32, tag="plog")
            for cls in range(WAY):
                nc.vector.tensor_reduce(plog[:, cls:cls + 1],
                                        mpsum[:, :, cls], axis=AXX, op=ADD)
            louts = apool.tile([NQC, WAY], F32, tag="louts")
            nc.scalar.activation(louts[:, :], plog[:, :], COPY,
                                 scale=-1.0 / (T * SCL))
            nc.sync.dma_start(out_d, louts[:, :])
    nc.compile()
    return nc


_NC_CACHE = {}
LAST = None


def _frames_fp8(x, n, frame_major):
    """[n, SEQ, D] f32 -> [128, KC, 2, cols] fp8 (DR moving layout)."""
    f8 = ml_dtypes.float8_e4m3
    if frame_major:
        fr = x.transpose(1, 0, 2).reshape(SEQ * n, KC, 2, 128)
    else:
        fr = x.reshape(n * SEQ, KC, 2, 128)
    return np.ascontiguousarray(fr.transpose(3, 1, 2, 0).astype(f8))


def kernel(support_set, queries, support_labels, W, b):
    global LAST
    support_set = np.asarray(support_set, dtype=np.float32)
    queries = np.asarray(queries, dtype=np.float32)
    support_labels = np.asarray(support_labels)
    W = np.asarray(W, dtype=np.float32)
    b = np.asarray(b, dtype=np.float32)
    f8 = ml_dtypes.float8_e4m3

    # class-major support reorder (class blocks contiguous)
    perm = np.argsort(support_labels, kind="stable")
    counts = tuple(int((support_labels == c).sum()) for c in range(WAY))
    sf = _frames_fp8(support_set[perm], NS, frame_major=False)

    # W: [p, m, half, kc, pair, hcol], scaled into fp8 range
    wt = np.ascontiguousarray(
        (W * SCL).reshape(MC, 128, 2, KC, 2, 128)
        .transpose(5, 0, 2, 3, 4, 1).astype(f8))
    bt = np.ascontiguousarray((b * SCL).reshape(MC, 128).T)

    # segment matrix: query-tuple cols are tuple-major -> query = col % NQC
    seg = np.zeros((128, NTILE, NQC), dtype=np.float32)
    for t in range(NTILE):
        for r in range(128):
            seg[r, t, (128 * t + r) % NQC] = 1.0
    seg = seg.astype(ml_dtypes.bfloat16)
    idm = np.eye(128, dtype=np.float32).astype(ml_dtypes.bfloat16)

    in_maps = []
    for c in range(N_CORES):
        qfc = _frames_fp8(queries[c * NQC:(c + 1) * NQC], NQC,
                          frame_major=True)
        in_maps.append({"qf": qfc, "sf": sf, "w": wt, "b": bt, "seg": seg,
                        "idm": idm})

    if counts not in _NC_CACHE:
        _NC_CACHE[counts] = build_program(counts)
    res = run_bass_kernel_spmd(_NC_CACHE[counts], in_maps,
                               list(range(N_CORES)))
    LAST = res
    outs = [res.results[c]["out"] for c in range(N_CORES)]
    return np.concatenate(outs, axis=0)


if __name__ == "__main__":
    rng = np.random.default_rng(0)
    out = kernel(
        rng.standard_normal((NS, SEQ, D)).astype(np.float32),
        rng.standard_normal((NQ_TOT, SEQ, D)).astype(np.float32),
        (np.arange(NS) % WAY).astype(np.int32),
        (rng.standard_normal((H, 2 * D)) / np.sqrt(2 * D)).astype(np.float32),
        (rng.standard_normal(H) * 0.01).astype(np.float32),
    )
    print(out.shape, out[:2])

